# revision 12
# baseline (speedup 1.0000x reference)
"""Causal self-attention with RoPE on 8 TRN2 NeuronCores.

Sharding: pure data parallel over batch B=8 (one batch element per core,
weights replicated, no collectives).

Per-core dataflow (everything "transposed" so softmax reductions and biases
land on friendly axes):
  xT = x^T                          via PE transpose          [C, T]
  q^T,k^T = W_qk^T @ x + b          PE (W stationary)         [ch, T]
  v natural = x @ W_v + b_v         PE (xT stationary)        [T, ch]
  RoPE(q,k)                         PE rotation matmul + DVE  in place
  s^T = k @ q^T (per head)          PE, K=64                  [Tk, Tq]
  p = exp(s/8) (+causal via -1e5)   ACT exp -> bf16 es tiles
  [y'; r]^T = [v, 1]^T @ p          PE, K=128 accumulation    [65, Tq]
  1/r broadcast via PE selector mm, y^T = y' * (1/r)  on DVE
  out = y @ W_proj + b              PE (yT stationary)        [T, C]

Engine split: PE matmuls; ACT exp + psum->sbuf bias copies; DVE rope
combines + normalize muls + recip; Pool (gpsimd) xT/osb copies + rope t1
muls. Phase B is software-pipelined across head pairs (scores(hp+1) emitted
before att@V(hp)) so the PE never waits on ACT's exp.

Matmuls run in float32r (fp32 data, 12-bit-mantissa multiply) = 4x fp32
rate; attention weights (es) and v are bf16 (same PE rate, half SBUF).
"""
import sys

sys.path.insert(0, "/opt/trn_rl_repo")

import numpy as np

B, T, C = 8, 1024, 768
H, D = 12, 64
N_CORES = 8
KC = C // 128  # 6 K-chunks of the C contraction
NT = T // 128  # 8 T-chunks

_prog = None  # cached compiled Bass program
_EXP_FUNC = "Exp"  # timing experiments may override


def _emit_body(nc, tc, dr, phases=(1, 2, 3)):
    """Emit one full forward pass. dr = dict of DRAM tensors."""
    from concourse import mybir

    F32 = mybir.dt.float32
    F32R = mybir.dt.float32r
    BF16 = mybir.dt.bfloat16
    AFT = mybir.ActivationFunctionType

    with (
        tc.tile_pool(name="persist", bufs=1) as pp,
    ):
        # persistent tensors
        qkT = pp.tile([128, 12, T], BF16, tag="qkT")  # 0-5: q pairs, 6-11: k pairs
        v_sb = pp.tile([128, NT, H, 65], BF16, tag="v")  # v natural + ones col
        yT = pp.tile([128, KC, T], F32R, tag="yT")
        cos_sb = pp.tile([128, T], BF16, tag="cos")
        sin_sb = pp.tile([128, T], BF16, tag="sin")
        rt_sb = pp.tile([128, 128], BF16, tag="rt")
        idn_sb = pp.tile([128, 128], F32, tag="idn")
        mneg_sb = pp.tile([128, 128], BF16, tag="mneg")
        idnr_sb = pp.tile([128, 128], BF16, tag="idnr")
        bqk_sb = pp.tile([128, 12], F32, tag="bqk")
        bv_sb = pp.tile([1, C], F32R, tag="bv")
        bp_sb = pp.tile([1, C], F32R, tag="bp")
        ones_sb = pp.tile([1, 128], F32R, tag="ones")

        nc.sync.dma_start(out=idn_sb[:], in_=dr["idn"][:])
        nc.sync.dma_start(out=cos_sb[:], in_=dr["cosT"][:])
        nc.sync.dma_start(out=sin_sb[:], in_=dr["sinT"][:])
        nc.sync.dma_start(out=rt_sb[:], in_=dr["rt"][:])
        nc.sync.dma_start(out=mneg_sb[:], in_=dr["mnegb"][:])
        nc.sync.dma_start(out=idnr_sb[:], in_=dr["idnb"][:])
        nc.sync.dma_start(out=bqk_sb[:], in_=dr["bqk"][:])
        nc.sync.dma_start(out=bv_sb[:], in_=dr["bv"][:].bitcast(F32R))
        nc.sync.dma_start(out=bp_sb[:], in_=dr["bp"][:].bitcast(F32R))
        onesF = pp.tile([128, 128], F32, tag="onesF")
        nc.vector.memset(onesF[:], 1.0)
        nc.vector.tensor_copy(ones_sb[:], onesF[0:1, :])
        for t in range(NT):
            nc.vector.tensor_copy(
                v_sb[:, t, :, 64:65],
                onesF[:, 0:12].rearrange("p (h o) -> p h o", h=12),
            )

        # ---------------- Phase A: transpose x, qkv, rope ----------------
        if 1 not in phases:
            return
        with (
            tc.tile_pool(name="pa_sb", bufs=2) as pa,
            tc.tile_pool(name="pa_xt", bufs=1) as paxt,
            tc.tile_pool(name="pa_ps", bufs=2, space="PSUM") as pap,
            tc.tile_pool(name="pa_mm", bufs=3, space="PSUM") as pam,
            tc.tile_pool(name="pa_tmp", bufs=3) as pat,
        ):
            xT = paxt.tile([128, KC, T], F32R, tag="xT")
            for t in range(NT):
                xn = pa.tile([128, C], F32, tag="xn", bufs=4)
                nc.sync.dma_start(out=xn[:], in_=dr["x"][t * 128 : (t + 1) * 128, :])
                for c in range(KC):
                    ptr = pap.tile([128, 128], F32, tag="tr")
                    nc.tensor.transpose(
                        ptr[:], xn[:, c * 128 : (c + 1) * 128], idn_sb[:]
                    )
                    nc.scalar.activation(
                        xT[:, c, t * 128 : (t + 1) * 128], ptr[:], AFT.Identity
                    )

            def _rope(i):
                t1 = pat.tile([128, T], BF16, tag="t1", bufs=2)
                nc.vector.tensor_mul(t1[:], qkT[:, i, :], cos_sb[:])
                for pj in range(2):
                    w = slice(pj * 512, (pj + 1) * 512)
                    rp = pap.tile([128, 512], F32, tag="rot")
                    nc.tensor.matmul(
                        rp[:], rt_sb[:], qkT[:, i, w], start=True, stop=True
                    )
                    t2 = pat.tile([128, 512], BF16, tag="t2")
                    nc.vector.tensor_mul(t2[:], rp[:], sin_sb[:, w])
                    nc.vector.tensor_add(qkT[:, i, w], t1[:, w], t2[:])

            # qkv in 6 column groups of 384 (W_attn streamed per group)
            wa_r = dr["wa"][:].bitcast(F32R).rearrange("(kc p) n -> p kc n", p=128)
            for g in range(6):
                wt = pa.tile([128, KC, 384], F32R, tag="walt")
                nc.sync.dma_start(out=wt[:], in_=wa_r[:, :, g * 384 : (g + 1) * 384])
                if g < 4:  # q/k output chunks m = 3g..3g+2
                    for pj in range(2):
                        for mi in range(3):
                            m = 3 * g + mi
                            w = slice(pj * 512, (pj + 1) * 512)
                            ps = pam.tile([128, 512], F32, tag="mm")
                            for kc in range(KC):
                                nc.tensor.matmul(
                                    ps[:],
                                    wt[:, kc, mi * 128 : (mi + 1) * 128],
                                    xT[:, kc, w],
                                    start=(kc == 0),
                                    stop=(kc == KC - 1),
                                )
                            nc.scalar.activation(
                                qkT[:, m, w],
                                ps[:],
                                AFT.Identity,
                                bias=bqk_sb[:, m : m + 1],
                            )
                    for mi in range(3):
                        _rope(3 * g + mi)
                else:  # v columns: 384-wide piece covers 6 heads
                    vg = g - 4
                    h0 = 6 * vg
                    for t in range(NT):
                        ps = pam.tile([128, 384], F32, tag="mm")
                        for kc in range(KC):
                            nc.tensor.matmul(
                                ps[:],
                                xT[:, kc, t * 128 : (t + 1) * 128],
                                wt[:, kc, :],
                                start=(kc == 0),
                                stop=False,
                            )
                        nc.tensor.matmul(
                            ps[:],
                            ones_sb[:],
                            bv_sb[:, vg * 384 : (vg + 1) * 384],
                            start=False,
                            stop=True,
                        )
                        nc.vector.tensor_copy(
                            v_sb[:, t, h0 : h0 + 6, 0:64],
                            ps[:].rearrange("p (h d) -> p h d", h=6),
                        )

        # ---------------- Phase B: attention per head ----------------
        if 2 not in phases:
            return
        bc_pool_cm = tc.tile_pool(name="pbc_wp", bufs=1)
        bc_pool = bc_pool_cm.__enter__()
        wp = bc_pool.tile([128, KC, C], F32R, tag="wp")
        nc.sync.dma_start(
            out=wp[:],
            in_=dr["wp"][:].bitcast(F32R).rearrange("(kc p) n -> p kc n", p=128),
        )
        with (
            tc.tile_pool(name="pb_es", bufs=32) as pbe,
            tc.tile_pool(name="pb_sc", bufs=4) as pbs,
            tc.tile_pool(name="pb_st", bufs=2, space="PSUM") as pbst,
            tc.tile_pool(name="pb_yp", bufs=3, space="PSUM") as pbyp,
        ):
            all_es = {}

            def scores(hp):
                """Emit s^T matmuls + exp for head pair hp; fill all_es."""
                qv, kv = hp, 6 + hp
                for tkc in range(NT):
                    lo = 128 * tkc
                    width = T - lo
                    for hh in range(2):  # adjacent K=64 MMs -> row-group overlap
                        b0 = 64 * hh
                        st = pbst.tile([128, 1024], F32, tag="st")
                        off = 0
                        while off < width:
                            wdt = min(512, width - off)
                            nc.tensor.matmul(
                                st[:, off : off + wdt],
                                qkT[b0 : b0 + 64, kv, lo : lo + 128],
                                qkT[b0 : b0 + 64, qv, lo + off : lo + off + wdt],
                                start=True,
                                stop=not (off == 0),
                            )
                            if off == 0:
                                nc.tensor.matmul(
                                    st[:, 0:128],
                                    idnr_sb[:],
                                    mneg_sb[:],
                                    start=False,
                                    stop=True,
                                )
                            off += wdt
                        es = pbe.tile([128, 1024], BF16, tag="es")
                        nc.scalar.activation(
                            es[:, :width],
                            st[:, :width],
                            getattr(AFT, _EXP_FUNC),
                            scale=0.125,
                        )
                        all_es[(hp, hh, tkc)] = es

            def attv(hp):
                """att@V + normalize for head pair hp (consumes all_es)."""
                for pj in range(2):
                    w0 = 512 * pj
                    tkcs = [k for k in range(NT) if 128 * k < w0 + 512]
                    rmat = pbs.tile([65, 1024], F32, tag="rmat", bufs=3)
                    yps = {}
                    for hh in range(2):
                        h = 2 * hp + hh
                        yp = pbyp.tile([65, 512], F32, tag="yp")
                        yps[hh] = yp
                        for j, tkc in enumerate(tkcs):
                            lo = 128 * tkc
                            plo = max(w0, lo)
                            wdt = w0 + 512 - plo
                            es = all_es[(hp, hh, tkc)]
                            nc.tensor.matmul(
                                yp[:, plo - w0 : plo - w0 + wdt],
                                v_sb[:, tkc, h, :],
                                es[:, plo - lo : plo - lo + wdt],
                                start=(j == 0),
                                stop=(j == len(tkcs) - 1),
                            )
                        nc.vector.tensor_copy(
                            rmat[64:65, 512 * hh : 512 * hh + 512], yp[64:65, :]
                        )
                    # shift r row to partition 0 via DMA, 1/r there, then
                    # broadcast to 64 partitions on gpsimd
                    rlow = pbs.tile([1, 1024], F32, tag="rl", bufs=3)
                    nc.sync.dma_start(out=rlow[0:1, :], in_=rmat[64:65, :])
                    rrec = pbs.tile([1, 1024], F32, tag="rc", bufs=3)
                    nc.vector.reciprocal_approx_fast(
                        out=rrec[0:1, :], in_=rlow[0:1, :]
                    )
                    bc = pbs.tile([64, 1024], F32, tag="bc", bufs=2)
                    nc.gpsimd.partition_broadcast(
                        out_ap=bc[:, :], in_ap=rrec[0:1, :]
                    )
                    nc.vector.tensor_mul(
                        yT[0:64, hp, w0 : w0 + 512], yps[0][0:64, :], bc[:, 0:512]
                    )
                    ys = pbs.tile([64, 512], F32R, tag="ys", bufs=3)
                    nc.vector.tensor_mul(ys[:], yps[1][0:64, :], bc[:, 512:1024])
                    nc.sync.dma_start(
                        out=yT[64:128, hp, w0 : w0 + 512], in_=ys[:]
                    )

            # software pipeline: scores(hp+1) emitted before att@V(hp)
            scores(0)
            for hp in range(1, 6):
                scores(hp)
                attv(hp - 1)
                for hh in range(2):
                    for tkc in range(NT):
                        del all_es[(hp - 1, hh, tkc)]
            attv(5)

        # ---------------- Phase C: output projection ----------------
        if 3 not in phases:
            bc_pool_cm.__exit__(None, None, None)
            return
        with (
            tc.tile_pool(name="pc_ob", bufs=3) as pco,
            tc.tile_pool(name="pc_ps", bufs=3, space="PSUM") as pcp,
        ):
            for m in range(NT):
                osb = pco.tile([128, C], F32, tag="ob")
                for piece in range(2):
                    pw = slice(piece * 384, (piece + 1) * 384)
                    po = pcp.tile([128, 384], F32, tag="po")
                    for kc in range(KC):
                        nc.tensor.matmul(
                            po[:],
                            yT[:, kc, m * 128 : (m + 1) * 128],
                            wp[:, kc, pw],
                            start=(kc == 0),
                            stop=False,
                        )
                    nc.tensor.matmul(
                        po[:], ones_sb[:], bp_sb[:, pw], start=False, stop=True
                    )
                    nc.scalar.activation(osb[:, pw], po[:], AFT.Identity)
                nc.sync.dma_start(out=dr["out"][m * 128 : (m + 1) * 128, :], in_=osb[:])
        bc_pool_cm.__exit__(None, None, None)


def _build_program(loop_n=None, phases=(1, 2, 3)):
    import concourse.bacc as bacc
    import concourse.tile as tile
    from concourse import mybir

    F32 = mybir.dt.float32

    nc = bacc.Bacc(None, target_bir_lowering=False, debug=False)

    dr = {
        "x": nc.dram_tensor("x", [T, C], F32, kind="ExternalInput"),
        "wa": nc.dram_tensor("wa", [C, 3 * C], F32, kind="ExternalInput"),
        "bqk": nc.dram_tensor("bqk", [128, 12], F32, kind="ExternalInput"),
        "bv": nc.dram_tensor("bv", [1, C], F32, kind="ExternalInput"),
        "wp": nc.dram_tensor("wp", [C, C], F32, kind="ExternalInput"),
        "bp": nc.dram_tensor("bp", [1, C], F32, kind="ExternalInput"),
        "cosT": nc.dram_tensor("cosT", [128, T], mybir.dt.bfloat16, kind="ExternalInput"),
        "sinT": nc.dram_tensor("sinT", [128, T], mybir.dt.bfloat16, kind="ExternalInput"),
        "rt": nc.dram_tensor("rt", [128, 128], mybir.dt.bfloat16, kind="ExternalInput"),
        "idn": nc.dram_tensor("idn", [128, 128], F32, kind="ExternalInput"),
        "mnegb": nc.dram_tensor("mnegb", [128, 128], mybir.dt.bfloat16, kind="ExternalInput"),
        "idnb": nc.dram_tensor("idnb", [128, 128], mybir.dt.bfloat16, kind="ExternalInput"),
        "out": nc.dram_tensor("out", [T, C], F32, kind="ExternalOutput"),
    }

    with tile.TileContext(nc) as tc:
        if loop_n is None:
            _emit_body(nc, tc, dr, phases)
        else:
            with tc.For_i(0, loop_n, 1):
                _emit_body(nc, tc, dr, phases)

    nc.compile()
    return nc


def _host_constants():
    """Constant tables shipped to every core."""
    inv_freq = (1.0 / (10000.0 ** (np.arange(0, D, 2, dtype=np.float32) / D))).astype(
        np.float32
    )
    tpos = np.arange(T, dtype=np.float32)
    freqs = tpos[None, :] * inv_freq[:, None]  # [32, T]
    cos32 = np.cos(freqs).astype(np.float32)
    sin32 = np.sin(freqs).astype(np.float32)
    cosT = np.repeat(cos32, 2, axis=0)  # [64, T], channel d -> freq d//2
    sinT = np.repeat(sin32, 2, axis=0)
    cosT = np.concatenate([cosT, cosT], axis=0)  # [128, T]: two head copies
    sinT = np.concatenate([sinT, sinT], axis=0)

    # rotation matrix: rot = R @ q with rot[2i] = -q[2i+1], rot[2i+1] = q[2i]
    R = np.zeros((128, 128), dtype=np.float32)
    idx = np.arange(0, 128, 2)
    R[idx, idx + 1] = -1.0
    R[idx + 1, idx] = 1.0
    RT = np.ascontiguousarray(R.T)

    idn = np.eye(128, dtype=np.float32)
    # additive mask: -1e5 (pre-scale) where tq_rel < tk so exp(0.125*s) == 0
    mneg = (-1.0e5 * np.tril(np.ones((128, 128), dtype=np.float32), k=-1)).astype(np.float32)
    import ml_dtypes
    mneg_b = mneg.astype(ml_dtypes.bfloat16)
    idn_b = idn.astype(ml_dtypes.bfloat16)
    cosT = cosT.astype(ml_dtypes.bfloat16)
    sinT = sinT.astype(ml_dtypes.bfloat16)
    RT = RT.astype(ml_dtypes.bfloat16)
    return cosT, sinT, RT, idn, mneg_b, idn_b


def _input_maps(x, W_attn, b_attn, W_proj, b_proj):
    cosT, sinT, RT, idn, mneg_b, idn_b = _host_constants()
    shared = {
        "wa": np.ascontiguousarray(W_attn),
        "bqk": np.ascontiguousarray(b_attn[: 2 * C].reshape(12, 128).T),
        "bv": np.ascontiguousarray(b_attn[2 * C :].reshape(1, C)),
        "wp": np.ascontiguousarray(W_proj),
        "bp": np.ascontiguousarray(b_proj.reshape(1, C)),
        "cosT": cosT,
        "sinT": sinT,
        "rt": RT,
        "idn": idn,
        "mnegb": mneg_b,
        "idnb": idn_b,
    }
    return [dict(shared, x=np.ascontiguousarray(x[b])) for b in range(B)]


def kernel(x, W_attn, b_attn, W_proj, b_proj):
    global _prog
    from concourse.bass_utils import run_bass_kernel_spmd

    if _prog is None:
        _prog = _build_program()

    x = np.asarray(x, dtype=np.float32)
    W_attn = np.asarray(W_attn, dtype=np.float32)
    b_attn = np.asarray(b_attn, dtype=np.float32)
    W_proj = np.asarray(W_proj, dtype=np.float32)
    b_proj = np.asarray(b_proj, dtype=np.float32)

    in_maps = _input_maps(x, W_attn, b_attn, W_proj, b_proj)
    res = run_bass_kernel_spmd(_prog, in_maps, list(range(N_CORES)))
    out = np.stack([res.results[b]["out"] for b in range(B)], axis=0)
    return out.astype(np.float32)


# revision 14
# speedup vs baseline: 1.2928x; 1.2928x over previous
"""Causal self-attention with RoPE on 8 TRN2 NeuronCores.

Sharding: pure data parallel over batch B=8 (one batch element per core,
weights replicated, no collectives).

Per-core dataflow (transposed so softmax reductions land on friendly axes):
  x -> bf16 (cast DMA), xT via DMA-transpose (xbar)     [C, T] bf16
  q^T,k^T = W_qk^T @ x + b     PE bf16 (W stationary)   [ch, T]
  v natural = x @ W_v + b_v    PE bf16 (xT stationary)  [T, ch]
  RoPE(q,k)                    PE rotation mm + DVE     in place, bf16
  s^T = k @ q^T (per head)     PE bf16, K=64            [Tk, Tq]
  p = exp(s/8) (+causal -1e5)  ACT exp -> bf16 es
  [y'; r]^T = [v,1]^T @ p      PE bf16, K=128 accum     [65, Tq]
  y^T = y' * (1/r)             DVE muls; 1/r broadcast via DMA shift +
                               gpsimd partition_broadcast
  out = y @ W_proj + b         PE f32r (yT stationary)  [T, C]

Phase A (qkv+rope) and phase B (attention) are stitched: the v-projection
groups run on PE while the first head pairs' exp runs on ACT, and phase B is
software-pipelined across head pairs (scores(hp+1) before att@V(hp)).
"""
import sys

sys.path.insert(0, "/opt/trn_rl_repo")

import numpy as np

B, T, C = 8, 1024, 768
H, D = 12, 64
N_CORES = 8
KC = C // 128  # 6 K-chunks of the C contraction
NT = T // 128  # 8 T-chunks

_prog = None  # cached compiled Bass program
_EXP_FUNC = "Exp"  # timing experiments may override


def _emit_body(nc, tc, dr, phases=(1, 2, 3)):
    """Emit one full forward pass. dr = dict of DRAM tensors."""
    from concourse import mybir

    F32 = mybir.dt.float32
    F32R = mybir.dt.float32r
    BF16 = mybir.dt.bfloat16
    AFT = mybir.ActivationFunctionType

    with (
        tc.tile_pool(name="persist", bufs=1) as pp,
    ):
        # persistent tensors
        qkT = pp.tile([128, 12, T], BF16, tag="qkT")  # 0-5: q pairs, 6-11: k
        v_sb = pp.tile([128, NT, H, 65], BF16, tag="v")  # v natural + ones col
        yT = pp.tile([128, KC, T], F32R, tag="yT")
        cos_sb = pp.tile([128, T], BF16, tag="cos")
        sin_sb = pp.tile([128, T], BF16, tag="sin")
        rt_sb = pp.tile([128, 128], BF16, tag="rt")
        mneg_sb = pp.tile([128, 128], BF16, tag="mneg")
        idnr_sb = pp.tile([128, 128], BF16, tag="idnr")
        bqk_sb = pp.tile([128, 12], F32, tag="bqk")
        bv_sb = pp.tile([1, C], F32R, tag="bv")
        bp_sb = pp.tile([1, C], F32R, tag="bp")
        ones_sb = pp.tile([1, 128], F32R, tag="ones")

        nc.sync.dma_start(out=cos_sb[:], in_=dr["cosT"][:])
        nc.sync.dma_start(out=sin_sb[:], in_=dr["sinT"][:])
        nc.sync.dma_start(out=rt_sb[:], in_=dr["rt"][:])
        nc.sync.dma_start(out=mneg_sb[:], in_=dr["mnegb"][:])
        nc.sync.dma_start(out=idnr_sb[:], in_=dr["idnb"][:])
        nc.sync.dma_start(out=bqk_sb[:], in_=dr["bqk"][:])
        nc.sync.dma_start(out=bv_sb[:], in_=dr["bv"][:].bitcast(F32R))
        nc.sync.dma_start(out=bp_sb[:], in_=dr["bp"][:].bitcast(F32R))
        onesF = pp.tile([128, 128], F32, tag="onesF")
        nc.vector.memset(onesF[:], 1.0)
        nc.vector.tensor_copy(ones_sb[:], onesF[0:1, :])
        for t in range(NT):
            nc.vector.tensor_copy(
                v_sb[:, t, :, 64:65],
                onesF[:, 0:12].rearrange("p (h o) -> p h o", h=12),
            )

        if 1 not in phases:
            return

        # ------------- Phase A: cast + transpose x, load W -------------
        paxt_cm = tc.tile_pool(name="pa_xt", bufs=1, side="right")
        paxt = paxt_cm.__enter__()
        pawt_cm = tc.tile_pool(name="pa_wt", bufs=1, side="right")
        pawt = pawt_cm.__enter__()
        xT = paxt.tile([128, KC, T], BF16, tag="xT")
        wt = pawt.tile([128, KC, 3 * C], BF16, tag="wt")
        nc.sync.dma_start(out=wt[:], in_=dr["wa"][:])
        with tc.tile_pool(name="pa_xb", bufs=1) as paxb:
            xb = paxb.tile([128, NT, C], BF16, tag="xb")
            for t in range(NT):
                nc.gpsimd.dma_start(
                    out=xb[:, t, :], in_=dr["x"][t * 128 : (t + 1) * 128, :]
                )
                nc.sync.dma_start_transpose(
                    out=xT[:, :, t * 128 : (t + 1) * 128], in_=xb[:, t, :]
                )

        # ------------- Phase A: q/k projections + rope -------------
        pamm_cm = tc.tile_pool(name="pa_mm", bufs=3, space="PSUM", side="right")
        pam = pamm_cm.__enter__()
        with (
            tc.tile_pool(name="pa_rot", bufs=2, space="PSUM") as parot,
            tc.tile_pool(name="pa_tmp", bufs=3) as pat,
        ):

            def _rope(i):
                t1 = pat.tile([128, T], BF16, tag="t1", bufs=2)
                nc.vector.tensor_mul(t1[:], qkT[:, i, :], cos_sb[:])
                for pj in range(2):
                    w = slice(pj * 512, (pj + 1) * 512)
                    rp = parot.tile([128, 512], F32, tag="rot")
                    nc.tensor.matmul(
                        rp[:], rt_sb[:], qkT[:, i, w], start=True, stop=True
                    )
                    t2 = pat.tile([128, 512], BF16, tag="t2")
                    nc.vector.tensor_mul(t2[:], rp[:], sin_sb[:, w])
                    nc.vector.tensor_add(qkT[:, i, w], t1[:, w], t2[:])

            for g in range(4):  # q/k output chunks m = 3g..3g+2
                for pj in range(2):
                    for mi in range(3):
                        m = 3 * g + mi
                        w = slice(pj * 512, (pj + 1) * 512)
                        ps = pam.tile([128, 512], F32, tag="mm")
                        for kc in range(KC):
                            nc.tensor.matmul(
                                ps[:],
                                wt[:, kc, g * 384 + mi * 128 : g * 384 + (mi + 1) * 128],
                                xT[:, kc, w],
                                start=(kc == 0),
                                stop=(kc == KC - 1),
                            )
                        nc.scalar.activation(
                            qkT[:, m, w],
                            ps[:],
                            AFT.Identity,
                            bias=bqk_sb[:, m : m + 1],
                        )
                for mi in range(3):
                    _rope(3 * g + mi)

        if 2 not in phases:
            pamm_cm.__exit__(None, None, None)
            pawt_cm.__exit__(None, None, None)
            paxt_cm.__exit__(None, None, None)
            return

        # ------------- Phase A tail (v proj) stitched with phase B -------------
        pbe_cm = tc.tile_pool(name="pb_es", bufs=32)
        pbe = pbe_cm.__enter__()
        pbst_cm = tc.tile_pool(name="pb_st", bufs=2, space="PSUM")
        pbst = pbst_cm.__enter__()

        all_es = {}

        def scores(hp):
            """Emit s^T matmuls + exp for head pair hp; fill all_es."""
            qv, kv = hp, 6 + hp
            for tkc in range(NT):
                lo = 128 * tkc
                width = T - lo
                for hh in range(2):  # adjacent K=64 MMs -> row-group overlap
                    b0 = 64 * hh
                    st = pbst.tile([128, 1024], F32, tag="st")
                    off = 0
                    while off < width:
                        wdt = min(512, width - off)
                        nc.tensor.matmul(
                            st[:, off : off + wdt],
                            qkT[b0 : b0 + 64, kv, lo : lo + 128],
                            qkT[b0 : b0 + 64, qv, lo + off : lo + off + wdt],
                            start=True,
                            stop=not (off == 0),
                        )
                        if off == 0:
                            nc.tensor.matmul(
                                st[:, 0:128],
                                idnr_sb[:],
                                mneg_sb[:],
                                start=False,
                                stop=True,
                            )
                        off += wdt
                    es = pbe.tile([128, 1024], BF16, tag="es")
                    nc.scalar.activation(
                        es[:, :width],
                        st[:, :width],
                        getattr(AFT, _EXP_FUNC),
                        scale=0.125,
                    )
                    all_es[(hp, hh, tkc)] = es

        def vgroup(vg):
            """v projection for heads 6vg..6vg+5 (phase A tail)."""
            h0 = 6 * vg
            for t in range(NT):
                ps = pam.tile([128, 384], F32, tag="mm")
                for kc in range(KC):
                    nc.tensor.matmul(
                        ps[:],
                        xT[:, kc, t * 128 : (t + 1) * 128],
                        wt[:, kc, (4 + vg) * 384 : (5 + vg) * 384],
                        start=(kc == 0),
                        stop=False,
                    )
                nc.tensor.matmul(
                    ps[:],
                    ones_sb[:],
                    bv_sb[:, vg * 384 : (vg + 1) * 384],
                    start=False,
                    stop=True,
                )
                nc.vector.tensor_copy(
                    v_sb[:, t, h0 : h0 + 6, 0:64],
                    ps[:].rearrange("p (h d) -> p h d", h=6),
                )

        vgroup(0)
        scores(0)
        vgroup(1)
        scores(1)
        pamm_cm.__exit__(None, None, None)
        pawt_cm.__exit__(None, None, None)
        paxt_cm.__exit__(None, None, None)

        # ------------- Phase B main: att@V + normalize -------------
        pwp_cm = tc.tile_pool(name="pb_wp", bufs=1, side="right")
        pwp = pwp_cm.__enter__()
        wp = pwp.tile([128, KC, C], F32R, tag="wp")
        nc.sync.dma_start(
            out=wp[:],
            in_=dr["wp"][:].bitcast(F32R).rearrange("(kc p) n -> p kc n", p=128),
        )
        pbs_cm = tc.tile_pool(name="pb_sc", bufs=2)
        pbs = pbs_cm.__enter__()
        pbyp_cm = tc.tile_pool(name="pb_yp", bufs=3, space="PSUM")
        pbyp = pbyp_cm.__enter__()

        def attv(hp):
            """att@V + normalize for head pair hp (consumes all_es)."""
            for pj in range(2):
                w0 = 512 * pj
                tkcs = [k for k in range(NT) if 128 * k < w0 + 512]
                ycops = {}
                for hh in range(2):
                    h = 2 * hp + hh
                    yp = pbyp.tile([65, 512], F32, tag="yp")
                    for j, tkc in enumerate(tkcs):
                        lo = 128 * tkc
                        plo = max(w0, lo)
                        wdt = w0 + 512 - plo
                        es = all_es[(hp, hh, tkc)]
                        nc.tensor.matmul(
                            yp[:, plo - w0 : plo - w0 + wdt],
                            v_sb[:, tkc, h, :],
                            es[:, plo - lo : plo - lo + wdt],
                            start=(j == 0),
                            stop=(j == len(tkcs) - 1),
                        )
                    # copy y'+r off PSUM immediately so the next av chain can
                    # recycle the PSUM bank; normalize lazily from SBUF
                    ycop = pbs.tile([65, 512], F32, tag="ycop", bufs=5)
                    nc.vector.tensor_copy(ycop[:], yp[:])
                    ycops[hh] = ycop
                # gather r rows on partition 0, 1/r there, broadcast to 64
                rlow = pbs.tile([1, 1024], F32, tag="rl", bufs=3)
                for hh in range(2):
                    nc.sync.dma_start(
                        out=rlow[0:1, 512 * hh : 512 * hh + 512],
                        in_=ycops[hh][64:65, :],
                    )
                rrec = pbs.tile([1, 1024], F32, tag="rc", bufs=3)
                nc.vector.reciprocal_approx_fast(
                    out=rrec[0:1, :], in_=rlow[0:1, :]
                )
                bc = pbs.tile([64, 1024], F32, tag="bc", bufs=2)
                nc.gpsimd.partition_broadcast(
                    out_ap=bc[:, :], in_ap=rrec[0:1, :]
                )
                nc.vector.tensor_mul(
                    yT[0:64, hp, w0 : w0 + 512], ycops[0][0:64, :], bc[:, 0:512]
                )
                nc.vector.tensor_mul(
                    yT[64:128, hp, w0 : w0 + 512],
                    ycops[1][0:64, :],
                    bc[:, 512:1024],
                )

        # software pipeline: scores(hp+1) emitted before att@V(hp)
        attv(0)
        for hp in range(2, 6):
            scores(hp)
            attv(hp - 1)
        attv(5)

        pbyp_cm.__exit__(None, None, None)
        pbs_cm.__exit__(None, None, None)
        pbst_cm.__exit__(None, None, None)
        pbe_cm.__exit__(None, None, None)

        # ---------------- Phase C: output projection ----------------
        if 3 not in phases:
            pwp_cm.__exit__(None, None, None)
            return
        with (
            tc.tile_pool(name="pc_ob", bufs=3) as pco,
            tc.tile_pool(name="pc_ps", bufs=3, space="PSUM") as pcp,
        ):
            for m in range(NT):
                osb = pco.tile([128, C], F32, tag="ob")
                for piece in range(2):
                    pw = slice(piece * 384, (piece + 1) * 384)
                    po = pcp.tile([128, 384], F32, tag="po")
                    for kc in range(KC):
                        nc.tensor.matmul(
                            po[:],
                            yT[:, kc, m * 128 : (m + 1) * 128],
                            wp[:, kc, pw],
                            start=(kc == 0),
                            stop=False,
                        )
                    nc.tensor.matmul(
                        po[:], ones_sb[:], bp_sb[:, pw], start=False, stop=True
                    )
                    nc.scalar.activation(osb[:, pw], po[:], AFT.Identity)
                nc.sync.dma_start(out=dr["out"][m * 128 : (m + 1) * 128, :], in_=osb[:])
        pwp_cm.__exit__(None, None, None)


def _build_program(loop_n=None, phases=(1, 2, 3)):
    import concourse.bacc as bacc
    import concourse.tile as tile
    from concourse import mybir

    F32 = mybir.dt.float32
    BF16 = mybir.dt.bfloat16

    nc = bacc.Bacc(None, target_bir_lowering=False, debug=False)

    dr = {
        "x": nc.dram_tensor("x", [T, C], F32, kind="ExternalInput"),
        "wa": nc.dram_tensor("wa", [128, KC, 3 * C], BF16, kind="ExternalInput"),
        "bqk": nc.dram_tensor("bqk", [128, 12], F32, kind="ExternalInput"),
        "bv": nc.dram_tensor("bv", [1, C], F32, kind="ExternalInput"),
        "wp": nc.dram_tensor("wp", [C, C], F32, kind="ExternalInput"),
        "bp": nc.dram_tensor("bp", [1, C], F32, kind="ExternalInput"),
        "cosT": nc.dram_tensor("cosT", [128, T], BF16, kind="ExternalInput"),
        "sinT": nc.dram_tensor("sinT", [128, T], BF16, kind="ExternalInput"),
        "rt": nc.dram_tensor("rt", [128, 128], BF16, kind="ExternalInput"),
        "mnegb": nc.dram_tensor("mnegb", [128, 128], BF16, kind="ExternalInput"),
        "idnb": nc.dram_tensor("idnb", [128, 128], BF16, kind="ExternalInput"),
        "out": nc.dram_tensor("out", [T, C], F32, kind="ExternalOutput"),
    }

    with tile.TileContext(nc) as tc:
        if loop_n is None:
            _emit_body(nc, tc, dr, phases)
        else:
            with tc.For_i(0, loop_n, 1):
                _emit_body(nc, tc, dr, phases)

    nc.compile()
    return nc


def _host_constants():
    """Constant tables shipped to every core."""
    import ml_dtypes

    inv_freq = (1.0 / (10000.0 ** (np.arange(0, D, 2, dtype=np.float32) / D))).astype(
        np.float32
    )
    tpos = np.arange(T, dtype=np.float32)
    freqs = tpos[None, :] * inv_freq[:, None]  # [32, T]
    cos32 = np.cos(freqs).astype(np.float32)
    sin32 = np.sin(freqs).astype(np.float32)
    cosT = np.repeat(cos32, 2, axis=0)  # [64, T], channel d -> freq d//2
    sinT = np.repeat(sin32, 2, axis=0)
    cosT = np.concatenate([cosT, cosT], axis=0)  # [128, T]: two head copies
    sinT = np.concatenate([sinT, sinT], axis=0)

    # rotation matrix: rot = R @ q with rot[2i] = -q[2i+1], rot[2i+1] = q[2i]
    R = np.zeros((128, 128), dtype=np.float32)
    idx = np.arange(0, 128, 2)
    R[idx, idx + 1] = -1.0
    R[idx + 1, idx] = 1.0
    RT = np.ascontiguousarray(R.T)

    idn = np.eye(128, dtype=np.float32)
    # additive mask: -1e5 (pre-scale) where tq_rel < tk so exp(0.125*s) == 0
    mneg = (-1.0e5 * np.tril(np.ones((128, 128), dtype=np.float32), k=-1)).astype(
        np.float32
    )
    mneg_b = mneg.astype(ml_dtypes.bfloat16)
    idn_b = idn.astype(ml_dtypes.bfloat16)
    cosT = cosT.astype(ml_dtypes.bfloat16)
    sinT = sinT.astype(ml_dtypes.bfloat16)
    RT = RT.astype(ml_dtypes.bfloat16)
    return cosT, sinT, RT, mneg_b, idn_b


def _input_maps(x, W_attn, b_attn, W_proj, b_proj):
    cosT, sinT, RT, mneg_b, idn_b = _host_constants()
    import ml_dtypes

    wa16 = np.ascontiguousarray(
        W_attn.reshape(KC, 128, 3 * C).transpose(1, 0, 2)
    ).astype(ml_dtypes.bfloat16)
    shared = {
        "wa": wa16,
        "bqk": np.ascontiguousarray(b_attn[: 2 * C].reshape(12, 128).T),
        "bv": np.ascontiguousarray(b_attn[2 * C :].reshape(1, C)),
        "wp": np.ascontiguousarray(W_proj),
        "bp": np.ascontiguousarray(b_proj.reshape(1, C)),
        "cosT": cosT,
        "sinT": sinT,
        "rt": RT,
        "mnegb": mneg_b,
        "idnb": idn_b,
    }
    return [dict(shared, x=np.ascontiguousarray(x[b])) for b in range(B)]


def kernel(x, W_attn, b_attn, W_proj, b_proj):
    global _prog
    from concourse.bass_utils import run_bass_kernel_spmd

    if _prog is None:
        _prog = _build_program()

    x = np.asarray(x, dtype=np.float32)
    W_attn = np.asarray(W_attn, dtype=np.float32)
    b_attn = np.asarray(b_attn, dtype=np.float32)
    W_proj = np.asarray(W_proj, dtype=np.float32)
    b_proj = np.asarray(b_proj, dtype=np.float32)

    in_maps = _input_maps(x, W_attn, b_attn, W_proj, b_proj)
    res = run_bass_kernel_spmd(_prog, in_maps, list(range(N_CORES)))
    out = np.stack([res.results[b]["out"] for b in range(B)], axis=0)
    return out.astype(np.float32)


# revision 18
# speedup vs baseline: 1.3464x; 1.0415x over previous
"""Causal self-attention with RoPE on 8 TRN2 NeuronCores.

Sharding: pure data parallel over batch B=8 (one batch element per core,
weights replicated, no collectives).

Per-core dataflow (transposed so softmax reductions land on friendly axes):
  x -> bf16 (cast DMA), xT via DMA-transpose (xbar)     [C, T] bf16
  q^T,k^T = W_qk^T @ x + b     PE bf16 (W stationary)   [ch, T]
  v natural = x @ W_v + b_v    PE bf16 (xT stationary)  [T, ch]
  RoPE(q,k)                    PE rotation mm + DVE     in place, bf16
  s^T = k @ q^T (per head)     PE bf16, K=64            [Tk, Tq]
  p = exp(s/8) (+causal -1e5)  ACT exp -> bf16 es
  [y'; r]^T = [v,1]^T @ p      PE bf16, K=128 accum     [65, Tq]
  y^T = y' * (1/r)             DVE muls; 1/r broadcast via DMA shift +
                               gpsimd partition_broadcast
  out = y @ W_proj + b         PE f32r (yT stationary)  [T, C]

Phase A (qkv+rope) and phase B (attention) are stitched: the v-projection
groups run on PE while the first head pairs' exp runs on ACT, and phase B is
software-pipelined across head pairs (scores(hp+1) before att@V(hp)).
"""
import sys

sys.path.insert(0, "/opt/trn_rl_repo")

import numpy as np

B, T, C = 8, 1024, 768
H, D = 12, 64
N_CORES = 8
KC = C // 128  # 6 K-chunks of the C contraction
NT = T // 128  # 8 T-chunks

_prog = None  # cached compiled Bass program
_EXP_FUNC = "Exp"  # timing experiments may override


def _emit_body(nc, tc, dr, phases=(1, 2, 3)):
    """Emit one full forward pass. dr = dict of DRAM tensors."""
    from concourse import mybir

    F32 = mybir.dt.float32
    F32R = mybir.dt.float32r
    BF16 = mybir.dt.bfloat16
    AFT = mybir.ActivationFunctionType

    with (
        tc.tile_pool(name="persist", bufs=1) as pp,
    ):
        # persistent tensors
        qkT = pp.tile([128, 12, T], BF16, tag="qkT")  # 0-5: q pairs, 6-11: k
        v_sb = pp.tile([128, NT, H, 65], BF16, tag="v")  # v natural + ones col
        yT = pp.tile([128, KC, T], F32R, tag="yT")
        cos_sb = pp.tile([128, T], BF16, tag="cos")
        sin_sb = pp.tile([128, T], BF16, tag="sin")
        rt_sb = pp.tile([128, 128], BF16, tag="rt")
        mneg_sb = pp.tile([128, 128], BF16, tag="mneg")
        idnr_sb = pp.tile([128, 128], BF16, tag="idnr")
        bqk_sb = pp.tile([128, 12], F32, tag="bqk")
        bv_sb = pp.tile([1, C], F32R, tag="bv")
        bp_sb = pp.tile([1, C], F32R, tag="bp")
        ones_sb = pp.tile([1, 128], F32R, tag="ones")

        nc.sync.dma_start(out=cos_sb[:], in_=dr["cosT"][:])
        nc.sync.dma_start(out=sin_sb[:], in_=dr["sinT"][:])
        nc.sync.dma_start(out=rt_sb[:], in_=dr["rt"][:])
        nc.sync.dma_start(out=mneg_sb[:], in_=dr["mnegb"][:])
        nc.sync.dma_start(out=idnr_sb[:], in_=dr["idnb"][:])
        nc.sync.dma_start(out=bqk_sb[:], in_=dr["bqk"][:])
        nc.sync.dma_start(out=bv_sb[:], in_=dr["bv"][:].bitcast(F32R))
        nc.sync.dma_start(out=bp_sb[:], in_=dr["bp"][:].bitcast(F32R))
        onesF = pp.tile([128, 128], F32, tag="onesF")
        nc.vector.memset(onesF[:], 1.0)
        nc.vector.tensor_copy(ones_sb[:], onesF[0:1, :])
        for t in range(NT):
            nc.vector.tensor_copy(
                v_sb[:, t, :, 64:65],
                onesF[:, 0:12].rearrange("p (h o) -> p h o", h=12),
            )

        if 1 not in phases:
            return

        # ------------- Phase A: cast + transpose x, load W -------------
        paxt_cm = tc.tile_pool(name="pa_xt", bufs=1, side="right")
        paxt = paxt_cm.__enter__()
        pawt_cm = tc.tile_pool(name="pa_wt", bufs=1, side="right")
        pawt = pawt_cm.__enter__()
        xT = paxt.tile([128, KC, T], BF16, tag="xT")
        wt = pawt.tile([128, KC, 3 * C], BF16, tag="wt")
        nc.sync.dma_start(out=wt[:], in_=dr["wa"][:])
        with tc.tile_pool(name="pa_xb", bufs=1) as paxb:
            xb = paxb.tile([128, NT, C], BF16, tag="xb")
            for t in range(NT):
                nc.gpsimd.dma_start(
                    out=xb[:, t, :], in_=dr["x"][t * 128 : (t + 1) * 128, :]
                )
                nc.sync.dma_start_transpose(
                    out=xT[:, :, t * 128 : (t + 1) * 128], in_=xb[:, t, :]
                )

        # ------------- Phase A: q/k projections + rope -------------
        pamm_cm = tc.tile_pool(name="pa_mm", bufs=3, space="PSUM", side="right")
        pam = pamm_cm.__enter__()
        with (
            tc.tile_pool(name="pa_rot", bufs=2, space="PSUM") as parot,
            tc.tile_pool(name="pa_tmp", bufs=3) as pat,
        ):

            def _rope(i):
                t1 = pat.tile([128, T], BF16, tag="t1", bufs=2)
                nc.vector.tensor_mul(t1[:], qkT[:, i, :], cos_sb[:])
                for pj in range(2):
                    w = slice(pj * 512, (pj + 1) * 512)
                    rp = parot.tile([128, 512], F32, tag="rot")
                    nc.tensor.matmul(
                        rp[:], rt_sb[:], qkT[:, i, w], start=True, stop=True
                    )
                    t2 = pat.tile([128, 512], BF16, tag="t2")
                    nc.vector.tensor_mul(t2[:], rp[:], sin_sb[:, w])
                    nc.vector.tensor_add(qkT[:, i, w], t1[:, w], t2[:])

            for g in range(4):  # q/k output chunks m = 3g..3g+2
                for pj in range(2):
                    for mi in range(3):
                        m = 3 * g + mi
                        w = slice(pj * 512, (pj + 1) * 512)
                        ps = pam.tile([128, 512], F32, tag="mm")
                        for kc in range(KC):
                            nc.tensor.matmul(
                                ps[:],
                                wt[:, kc, g * 384 + mi * 128 : g * 384 + (mi + 1) * 128],
                                xT[:, kc, w],
                                start=(kc == 0),
                                stop=(kc == KC - 1),
                            )
                        nc.scalar.activation(
                            qkT[:, m, w],
                            ps[:],
                            AFT.Identity,
                            bias=bqk_sb[:, m : m + 1],
                        )
                for mi in range(3):
                    _rope(3 * g + mi)

        if 2 not in phases:
            pamm_cm.__exit__(None, None, None)
            pawt_cm.__exit__(None, None, None)
            paxt_cm.__exit__(None, None, None)
            return

        # ------------- Phase A tail (v proj) stitched with phase B -------------
        pbe_cm = tc.tile_pool(name="pb_es", bufs=32)
        pbe = pbe_cm.__enter__()
        pbst_cm = tc.tile_pool(name="pb_st", bufs=2, space="PSUM")
        pbst = pbst_cm.__enter__()

        all_es = {}

        def scores(hp):
            """Emit s^T matmuls + exp for head pair hp; fill all_es."""
            qv, kv = hp, 6 + hp
            for tkc in range(NT):
                lo = 128 * tkc
                width = T - lo
                for hh in range(2):  # adjacent K=64 MMs -> row-group overlap
                    b0 = 64 * hh
                    st = pbst.tile([128, 1024], F32, tag="st")
                    off = 0
                    while off < width:
                        wdt = min(512, width - off)
                        nc.tensor.matmul(
                            st[:, off : off + wdt],
                            qkT[b0 : b0 + 64, kv, lo : lo + 128],
                            qkT[b0 : b0 + 64, qv, lo + off : lo + off + wdt],
                            start=True,
                            stop=not (off == 0),
                        )
                        if off == 0:
                            nc.tensor.matmul(
                                st[:, 0:128],
                                idnr_sb[:],
                                mneg_sb[:],
                                start=False,
                                stop=True,
                            )
                        off += wdt
                    es = pbe.tile([128, 1024], BF16, tag="es")
                    nc.scalar.activation(
                        es[:, :width],
                        st[:, :width],
                        getattr(AFT, _EXP_FUNC),
                        scale=0.125,
                    )
                    all_es[(hp, hh, tkc)] = es

        def vgroup(vg):
            """v projection for heads 6vg..6vg+5 (phase A tail)."""
            h0 = 6 * vg
            for t in range(NT):
                ps = pam.tile([128, 384], F32, tag="mm")
                for kc in range(KC):
                    nc.tensor.matmul(
                        ps[:],
                        xT[:, kc, t * 128 : (t + 1) * 128],
                        wt[:, kc, (4 + vg) * 384 : (5 + vg) * 384],
                        start=(kc == 0),
                        stop=False,
                    )
                nc.tensor.matmul(
                    ps[:],
                    ones_sb[:],
                    bv_sb[:, vg * 384 : (vg + 1) * 384],
                    start=False,
                    stop=True,
                )
                nc.vector.tensor_copy(
                    v_sb[:, t, h0 : h0 + 6, 0:64],
                    ps[:].rearrange("p (h d) -> p h d", h=6),
                )

        vgroup(0)
        scores(0)
        vgroup(1)
        scores(1)
        pamm_cm.__exit__(None, None, None)
        pawt_cm.__exit__(None, None, None)
        paxt_cm.__exit__(None, None, None)

        # ------------- Phase B main: att@V + normalize -------------
        pwp_cm = tc.tile_pool(name="pb_wp", bufs=1, side="right")
        pwp = pwp_cm.__enter__()
        wp = pwp.tile([128, KC, C], F32R, tag="wp")
        nc.sync.dma_start(
            out=wp[:],
            in_=dr["wp"][:].bitcast(F32R).rearrange("(kc p) n -> p kc n", p=128),
        )
        pbs_cm = tc.tile_pool(name="pb_sc", bufs=2)
        pbs = pbs_cm.__enter__()
        pbyp_cm = tc.tile_pool(name="pb_yp", bufs=3, space="PSUM")
        pbyp = pbyp_cm.__enter__()

        def attv(hp):
            """att@V + normalize for head pair hp (consumes all_es)."""
            for pj in range(2):
                w0 = 512 * pj
                tkcs = [k for k in range(NT) if 128 * k < w0 + 512]
                ycops = {}
                for hh in range(2):
                    h = 2 * hp + hh
                    yp = pbyp.tile([65, 512], F32, tag="yp")
                    for j, tkc in enumerate(tkcs):
                        lo = 128 * tkc
                        plo = max(w0, lo)
                        wdt = w0 + 512 - plo
                        es = all_es[(hp, hh, tkc)]
                        nc.tensor.matmul(
                            yp[:, plo - w0 : plo - w0 + wdt],
                            v_sb[:, tkc, h, :],
                            es[:, plo - lo : plo - lo + wdt],
                            start=(j == 0),
                            stop=(j == len(tkcs) - 1),
                        )
                    # copy y'+r off PSUM immediately so the next av chain can
                    # recycle the PSUM bank; normalize lazily from SBUF
                    ycop = pbs.tile([65, 512], F32, tag="ycop", bufs=5)
                    nc.vector.tensor_copy(ycop[:], yp[:])
                    ycops[hh] = ycop
                # gather r rows on partition 0, 1/r there, broadcast to 64
                rlow = pbs.tile([1, 1024], F32, tag="rl", bufs=3)
                for hh in range(2):
                    nc.sync.dma_start(
                        out=rlow[0:1, 512 * hh : 512 * hh + 512],
                        in_=ycops[hh][64:65, :],
                    )
                rrec = pbs.tile([1, 1024], F32, tag="rc", bufs=3)
                nc.vector.reciprocal_approx_fast(
                    out=rrec[0:1, :], in_=rlow[0:1, :]
                )
                bc = pbs.tile([64, 1024], F32, tag="bc", bufs=2)
                nc.gpsimd.partition_broadcast(
                    out_ap=bc[:, :], in_ap=rrec[0:1, :]
                )
                nc.vector.tensor_mul(
                    yT[0:64, hp, w0 : w0 + 512], ycops[0][0:64, :], bc[:, 0:512]
                )
                nc.vector.tensor_mul(
                    yT[64:128, hp, w0 : w0 + 512],
                    ycops[1][0:64, :],
                    bc[:, 512:1024],
                )

        # software pipeline: scores(hp+1) emitted before att@V(hp)
        attv(0)
        for hp in range(2, 6):
            scores(hp)
            attv(hp - 1)
        attv(5)

        pbyp_cm.__exit__(None, None, None)
        pbs_cm.__exit__(None, None, None)
        pbst_cm.__exit__(None, None, None)
        pbe_cm.__exit__(None, None, None)

        # ---------------- Phase C: output projection ----------------
        if 3 not in phases:
            pwp_cm.__exit__(None, None, None)
            return
        with (
            tc.tile_pool(name="pc_ob", bufs=3) as pco,
            tc.tile_pool(name="pc_ps", bufs=3, space="PSUM") as pcp,
        ):
            for m in range(NT):
                osb = pco.tile([128, C], F32, tag="ob")
                for piece in range(2):
                    pw = slice(piece * 384, (piece + 1) * 384)
                    po = pcp.tile([128, 384], F32, tag="po")
                    for kc in range(KC):
                        nc.tensor.matmul(
                            po[:],
                            yT[:, kc, m * 128 : (m + 1) * 128],
                            wp[:, kc, pw],
                            start=(kc == 0),
                            stop=False,
                        )
                    nc.tensor.matmul(
                        po[:], ones_sb[:], bp_sb[:, pw], start=False, stop=True
                    )
                    nc.scalar.activation(osb[:, pw], po[:], AFT.Identity)
                nc.sync.dma_start(out=dr["out"][m * 128 : (m + 1) * 128, :], in_=osb[:])
        pwp_cm.__exit__(None, None, None)


def _build_program(loop_n=None, phases=(1, 2, 3)):
    import concourse.bacc as bacc
    import concourse.tile as tile
    from concourse import mybir

    F32 = mybir.dt.float32
    BF16 = mybir.dt.bfloat16

    nc = bacc.Bacc(None, target_bir_lowering=False, debug=False)

    dr = {
        "x": nc.dram_tensor("x", [T, C], F32, kind="ExternalInput"),
        "wa": nc.dram_tensor("wa", [128, KC, 3 * C], BF16, kind="ExternalInput"),
        "bqk": nc.dram_tensor("bqk", [128, 12], F32, kind="ExternalInput"),
        "bv": nc.dram_tensor("bv", [1, C], F32, kind="ExternalInput"),
        "wp": nc.dram_tensor("wp", [C, C], F32, kind="ExternalInput"),
        "bp": nc.dram_tensor("bp", [1, C], F32, kind="ExternalInput"),
        "cosT": nc.dram_tensor("cosT", [128, T], BF16, kind="ExternalInput"),
        "sinT": nc.dram_tensor("sinT", [128, T], BF16, kind="ExternalInput"),
        "rt": nc.dram_tensor("rt", [128, 128], BF16, kind="ExternalInput"),
        "mnegb": nc.dram_tensor("mnegb", [128, 128], BF16, kind="ExternalInput"),
        "idnb": nc.dram_tensor("idnb", [128, 128], BF16, kind="ExternalInput"),
        "out": nc.dram_tensor("out", [T, C], F32, kind="ExternalOutput"),
    }

    with tile.TileContext(nc) as tc:
        if loop_n is None:
            _emit_body(nc, tc, dr, phases)
        else:
            with tc.For_i(0, loop_n, 1):
                _emit_body(nc, tc, dr, phases)

    nc.compile()
    return nc


def _host_constants():
    """Constant tables shipped to every core."""
    import ml_dtypes

    inv_freq = (1.0 / (10000.0 ** (np.arange(0, D, 2, dtype=np.float32) / D))).astype(
        np.float32
    )
    tpos = np.arange(T, dtype=np.float32)
    freqs = tpos[None, :] * inv_freq[:, None]  # [32, T]
    cos32 = np.cos(freqs).astype(np.float32)
    sin32 = np.sin(freqs).astype(np.float32)
    cosT = np.repeat(cos32, 2, axis=0)  # [64, T], channel d -> freq d//2
    sinT = np.repeat(sin32, 2, axis=0)
    cosT = np.concatenate([cosT, cosT], axis=0)  # [128, T]: two head copies
    sinT = np.concatenate([sinT, sinT], axis=0)

    # rotation matrix: rot = R @ q with rot[2i] = -q[2i+1], rot[2i+1] = q[2i]
    R = np.zeros((128, 128), dtype=np.float32)
    idx = np.arange(0, 128, 2)
    R[idx, idx + 1] = -1.0
    R[idx + 1, idx] = 1.0
    RT = np.ascontiguousarray(R.T)

    idn = np.eye(128, dtype=np.float32)
    # additive mask: -1e5 (pre-scale) where tq_rel < tk so exp(0.125*s) == 0
    mneg = (-1.0e5 * np.tril(np.ones((128, 128), dtype=np.float32), k=-1)).astype(
        np.float32
    )
    mneg_b = mneg.astype(ml_dtypes.bfloat16)
    idn_b = idn.astype(ml_dtypes.bfloat16)
    cosT = cosT.astype(ml_dtypes.bfloat16)
    sinT = sinT.astype(ml_dtypes.bfloat16)
    RT = RT.astype(ml_dtypes.bfloat16)
    return cosT, sinT, RT, mneg_b, idn_b


def _input_maps(x, W_attn, b_attn, W_proj, b_proj):
    cosT, sinT, RT, mneg_b, idn_b = _host_constants()
    import ml_dtypes

    wa16 = np.ascontiguousarray(
        W_attn.reshape(KC, 128, 3 * C).transpose(1, 0, 2)
    ).astype(ml_dtypes.bfloat16)
    shared = {
        "wa": wa16,
        "bqk": np.ascontiguousarray(b_attn[: 2 * C].reshape(12, 128).T),
        "bv": np.ascontiguousarray(b_attn[2 * C :].reshape(1, C)),
        "wp": np.ascontiguousarray(W_proj),
        "bp": np.ascontiguousarray(b_proj.reshape(1, C)),
        "cosT": cosT,
        "sinT": sinT,
        "rt": RT,
        "mnegb": mneg_b,
        "idnb": idn_b,
    }
    return [dict(shared, x=np.ascontiguousarray(x[b])) for b in range(B)]


def kernel(x, W_attn, b_attn, W_proj, b_proj):
    global _prog
    from concourse.bass_utils import run_bass_kernel_spmd

    if _prog is None:
        _prog = _build_program()

    x = np.asarray(x, dtype=np.float32)
    W_attn = np.asarray(W_attn, dtype=np.float32)
    b_attn = np.asarray(b_attn, dtype=np.float32)
    W_proj = np.asarray(W_proj, dtype=np.float32)
    b_proj = np.asarray(b_proj, dtype=np.float32)

    in_maps = _input_maps(x, W_attn, b_attn, W_proj, b_proj)
    res = run_bass_kernel_spmd(_prog, in_maps, list(range(N_CORES)))
    out = np.stack([res.results[b]["out"] for b in range(B)], axis=0)
    return out.astype(np.float32)
